# revision 7
# baseline (speedup 1.0000x reference)
# Dense-MoE (all experts active) Trainium2 kernel, TOKEN-parallel (shard B)
# over 8 NeuronCores. Each core runs its 512-token shard through ALL 8
# experts and accumulates the gate-weighted expert sum locally:
#   out[b,:] = sum_e gelu(h[b] @ W1[e] + b1[e]) @ (p_e*W2[e]) + sum_e p_e*b2[e]
# No collectives at all: a NEFF with armed collectives caps the PE clock at
# ~2.0 GHz (P0 power state) chip-wide; collective-free kernels sustain
# 2.4 GHz. The host just concatenates the 8 row-shards.
#
# Per-core: 2048 matmuls [128x512] fp16 (same FLOPs as expert-parallel).
# Expert weights (8 MB/expert fp16) stream from HBM, double-buffered, one
# expert ahead (~146 GB/s sustained, well under the 358 GB/s per-NC limit).
# The expert sum lives in an SBUF f32 accumulator updated by in-place DVE
# adds from L2's PSUM; the final pass adds the folded b2 and casts to fp16.
import os
import sys

sys.path.insert(0, "/opt/trn_rl_repo")

import numpy as np

import concourse.mybir as mybir
from concourse import bacc, tile

B, E, IN, H, D = 4096, 8, 1024, 2048, 1024
NCORES = 8
P = 128
T = B // NCORES           # 512 tokens per core
NSUB = T // P             # 4 token sub-tiles
KC1 = IN // P             # 8 contraction chunks, layer 1
MC1 = H // P              # 16 H chunks
ND = D // 512             # 2 output column slices of 512

F32 = mybir.dt.float32
F16 = mybir.dt.float16

_CACHE = {}


def build():
    nc = bacc.Bacc("TRN2", target_bir_lowering=False)

    # layouts: x3[p, c, f] so each SBUF tile loads as one strided DMA
    ht3 = nc.declare_dram_parameter("ht3", [P, KC1, T], F16, isOutput=False)
    w13 = nc.declare_dram_parameter("w13", [P, E * KC1, H], F16,
                                    isOutput=False)
    b1t = nc.declare_dram_parameter("b1t", [P, E * MC1], F32, isOutput=False)
    w23 = nc.declare_dram_parameter("w23", [P, E * MC1, D], F16,
                                    isOutput=False)
    b2s = nc.declare_dram_parameter("b2s", [P, D], F32, isOutput=False)
    out = nc.declare_dram_parameter("out", [T, D], F16, isOutput=True)

    with tile.TileContext(nc) as tc:
        with (
            tc.tile_pool(name="w1p", bufs=2) as w1_pool,
            tc.tile_pool(name="w2p", bufs=2) as w2_pool,
            tc.tile_pool(name="hid", bufs=2) as hid_pool,
            tc.tile_pool(name="consts", bufs=1) as cpool,
            tc.tile_pool(name="osb", bufs=2) as out_pool,
            tc.tile_pool(name="fetmp", bufs=2) as fe_pool,
            tc.tile_pool(name="l1_ps", bufs=3, space="PSUM") as l1_psum,
            tc.tile_pool(name="l2_ps", bufs=4, space="PSUM") as l2_psum,
        ):
            ht = cpool.tile([P, KC1, T], F16, tag="ht")
            # fp16 ping-pong expert accumulators (DVE in-place add on one
            # buffer is illegal -> read acc[e%2], write acc[(e+1)%2])
            acc_a = cpool.tile([P, NSUB * D], F16, tag="acc_a")
            acc_b = cpool.tile([P, NSUB * D], F16, tag="acc_b")
            accs = [acc_a, acc_b]
            b1_sb = cpool.tile([P, E * MC1], F32, tag="b1")
            b2_sb = cpool.tile([P, D], F32, tag="b2")

            # expert 0 weights at slab granularity (fast head: first matmul
            # only needs slab k=0 + ht chunk 0), interleaved with ht chunks
            w1_t = [None] * E
            w2_t = [None] * E
            w1_t[0] = w1_pool.tile([P, KC1, H], F16, tag="w1", name="w1_0")
            for k in range(KC1):
                nc.sync.dma_start(ht[:, k:k + 1, :], ht3[:, k:k + 1, :])
                nc.sync.dma_start(w1_t[0][:, k:k + 1, :],
                                  w13[:, k:k + 1, :])
            nc.sync.dma_start(b1_sb[:], b1t[:])
            w2_t[0] = w2_pool.tile([P, MC1, D], F16, tag="w2", name="w2_0")
            for q in range(4):
                nc.sync.dma_start(
                    w2_t[0][:, 4 * q:4 * q + 4, :],
                    w23[:, 4 * q:4 * q + 4, :],
                )
            nc.sync.dma_start(b2_sb[:], b2s[:])

            def prefetch_w(e):
                w1_t[e] = w1_pool.tile([P, KC1, H], F16, tag="w1", name=f"w1_{e}")
                nc.sync.dma_start(w1_t[e][:],
                                  w13[:, e * KC1:(e + 1) * KC1, :])
                w2_t[e] = w2_pool.tile([P, MC1, D], F16, tag="w2", name=f"w2_{e}")
                nc.scalar.dma_start(w2_t[e][:],
                                    w23[:, e * MC1:(e + 1) * MC1, :])

            prefetch_w(1)

            for e in range(E):
                # --- layer 1: hidT chunk m = (W1 blk).T @ hT, +b1, gelu ---
                # Expert 0 runs k-outer over m-groups of 3 so the first
                # matmuls consume W1 slabs at their DMA arrival cadence
                # (m-outer would need all 8 slabs within the first chain).
                hid = hid_pool.tile([P, MC1, T], F16, tag="hid")

                def l1_act(m, ps):
                    nc.scalar.activation(
                        hid[:, m, :],
                        ps[:],
                        mybir.ActivationFunctionType.Gelu,
                        bias=b1_sb[:, e * MC1 + m:e * MC1 + m + 1],
                        scale=1.0,
                    )

                if e == 0:
                    for m0 in range(0, MC1, 3):
                        ms = list(range(m0, min(m0 + 3, MC1)))
                        pss_l1 = [
                            l1_psum.tile([P, T], F32, tag="l1",
                                         name=f"l1ps_{m0}_{i}")
                            for i in range(len(ms))]
                        for k in range(KC1):
                            for mi, m in enumerate(ms):
                                nc.tensor.matmul(
                                    pss_l1[mi][:],
                                    w1_t[e][:, k, m * P:(m + 1) * P],
                                    ht[:, k, :],
                                    start=(k == 0),
                                    stop=(k == KC1 - 1),
                                )
                        for mi, m in enumerate(ms):
                            l1_act(m, pss_l1[mi])
                else:
                    for m in range(MC1):
                        ps = l1_psum.tile([P, T], F32, tag="l1")
                        for k in range(KC1):
                            nc.tensor.matmul(
                                ps[:],
                                w1_t[e][:, k, m * P:(m + 1) * P],
                                ht[:, k, :],
                                start=(k == 0),
                                stop=(k == KC1 - 1),
                            )
                        l1_act(m, ps)

                if 1 <= e < E - 1:
                    prefetch_w(e + 1)

                # --- layer 2, accumulated into fe_acc over experts ---
                for s in range(NSUB):
                    ps_a = l2_psum.tile([P, 512], F32, tag="l2")
                    ps_b = l2_psum.tile([P, 512], F32, tag="l2")
                    pss = [ps_a, ps_b]
                    for m in range(MC1):
                        hs = hid[:, m, s * P:(s + 1) * P]
                        for d in range(ND):
                            mi = nc.tensor.matmul(
                                pss[d][:],
                                hs,
                                w2_t[e][:, m, d * 512:(d + 1) * 512],
                                start=(m == 0),
                                stop=(m == MC1 - 1),
                            )
                            if d > 0:
                                mi.ins.ldweights = False
                    if e == E - 1:
                        # last expert: single add (b2 was folded in at e=0),
                        # d-slices on separate engines, store immediately
                        osb = out_pool.tile([P, D], F16, tag="osb")
                        for d in range(ND):
                            lo = s * D + d * 512
                            nc.vector.tensor_add(
                                osb[:, d * 512:(d + 1) * 512], pss[d][:],
                                accs[e % 2][:, lo:lo + 512])
                        nc.sync.dma_start(out[s * P:(s + 1) * P, :], osb[:])
                    else:
                        for d in range(ND):
                            lo, hi = s * D + d * 512, s * D + (d + 1) * 512
                            if e == 0:
                                # fold the (gate-weighted) b2 in here so the
                                # final expert needs only one add
                                nc.vector.tensor_add(
                                    accs[1][:, lo:hi], pss[d][:],
                                    b2_sb[:, d * 512:(d + 1) * 512])
                            else:
                                nc.vector.tensor_add(
                                    accs[(e + 1) % 2][:, lo:hi], pss[d][:],
                                    accs[e % 2][:, lo:hi])

    nc.finalize()
    return nc


def _get_nc():
    if "nc" not in _CACHE:
        _CACHE["nc"] = build()
    return _CACHE["nc"]


def _run(inputs, trace=False):
    from concourse.bass_utils import run_bass_kernel_spmd

    h = np.asarray(inputs["h"], dtype=np.float32)
    gate_logits = np.asarray(inputs["gate_logits"], dtype=np.float64)
    W1 = np.asarray(inputs["W1"], dtype=np.float32)
    b1 = np.asarray(inputs["b1"], dtype=np.float32)
    W2 = np.asarray(inputs["W2"], dtype=np.float32)
    b2 = np.asarray(inputs["b2"], dtype=np.float32)

    # gate: softmax over E; fold into W2/b2
    z = np.exp(gate_logits - gate_logits.max())
    probs = (z / z.sum()).astype(np.float32)

    # w13[p, e*KC1+k, :] = W1[e, k*128+p, :]
    w13 = np.ascontiguousarray(
        W1.reshape(E * KC1, P, H).transpose(1, 0, 2).astype(np.float16)
    )
    w23 = np.ascontiguousarray(
        (W2 * probs[:, None, None]).reshape(E * MC1, P, D)
        .transpose(1, 0, 2).astype(np.float16)
    )
    # b1t[p, e, m] = b1[e, m*128+p]
    b1t = np.ascontiguousarray(
        b1.reshape(E, MC1, P).transpose(2, 0, 1).reshape(P, E * MC1)
        .astype(np.float32)
    )
    b2sum = (probs[:, None] * b2).sum(axis=0).astype(np.float32)
    b2s = np.ascontiguousarray(np.broadcast_to(b2sum, (P, D)))

    in_maps = []
    for r in range(NCORES):
        hr = h[r * T:(r + 1) * T, :]                      # [T, IN]
        ht3 = np.ascontiguousarray(
            hr.T.reshape(KC1, P, T).transpose(1, 0, 2).astype(np.float16)
        )
        in_maps.append(
            {"ht3": ht3, "w13": w13, "b1t": b1t, "w23": w23, "b2s": b2s}
        )

    nc = _get_nc()
    res = run_bass_kernel_spmd(nc, in_maps, list(range(NCORES)), trace=trace)

    final = np.concatenate(
        [np.asarray(res.results[r]["out"], dtype=np.float32)
         for r in range(NCORES)], axis=0
    )
    return final, res


def kernel(**inputs):
    final, _ = _run(inputs, trace=False)
    return final


# revision 8
# speedup vs baseline: 1.0409x; 1.0409x over previous
# Dense-MoE (all experts active) Trainium2 kernel, TOKEN-parallel (shard B)
# over 8 NeuronCores. Each core runs its 512-token shard through ALL 8
# experts and accumulates the gate-weighted expert sum locally:
#   out[b,:] = sum_e gelu(h[b] @ W1[e] + b1[e]) @ (p_e*W2[e]) + sum_e p_e*b2[e]
# No collectives at all: a NEFF with armed collectives caps the PE clock at
# ~2.0 GHz (P0 power state) chip-wide; collective-free kernels sustain
# 2.4 GHz. The host just concatenates the 8 row-shards.
#
# Per-core: 2048 matmuls [128x512] fp16 (same FLOPs as expert-parallel).
# Expert weights (8 MB/expert fp16) stream from HBM, double-buffered, one
# expert ahead (~146 GB/s sustained, well under the 358 GB/s per-NC limit).
# The expert sum lives in an SBUF f32 accumulator updated by in-place DVE
# adds from L2's PSUM; the final pass adds the folded b2 and casts to fp16.
import os
import sys

sys.path.insert(0, "/opt/trn_rl_repo")

import numpy as np

import concourse.mybir as mybir
from concourse import bacc, tile

B, E, IN, H, D = 4096, 8, 1024, 2048, 1024
NCORES = 8
P = 128
T = B // NCORES           # 512 tokens per core
NSUB = T // P             # 4 token sub-tiles
KC1 = IN // P             # 8 contraction chunks, layer 1
MC1 = H // P              # 16 H chunks
ND = D // 512             # 2 output column slices of 512

F32 = mybir.dt.float32
F16 = mybir.dt.float16

_CACHE = {}


def build():
    nc = bacc.Bacc("TRN2", target_bir_lowering=False)

    # layouts: x3[p, c, f] so each SBUF tile loads as one strided DMA
    ht3 = nc.declare_dram_parameter("ht3", [P, KC1, T], F16, isOutput=False)
    w13 = nc.declare_dram_parameter("w13", [P, E * KC1, H], F16,
                                    isOutput=False)
    b1t = nc.declare_dram_parameter("b1t", [P, E * MC1], F32, isOutput=False)
    w23 = nc.declare_dram_parameter("w23", [P, E * MC1, D], F16,
                                    isOutput=False)
    b2s = nc.declare_dram_parameter("b2s", [P, D], F32, isOutput=False)
    out = nc.declare_dram_parameter("out", [T, D], F16, isOutput=True)

    with tile.TileContext(nc) as tc:
        with (
            tc.tile_pool(name="w1p", bufs=2) as w1_pool,
            tc.tile_pool(name="w2p", bufs=2) as w2_pool,
            tc.tile_pool(name="hid", bufs=2) as hid_pool,
            tc.tile_pool(name="consts", bufs=1) as cpool,
            tc.tile_pool(name="osb", bufs=2) as out_pool,
            tc.tile_pool(name="fetmp", bufs=2) as fe_pool,
            tc.tile_pool(name="l1_ps", bufs=3, space="PSUM") as l1_psum,
            tc.tile_pool(name="l2_ps", bufs=4, space="PSUM") as l2_psum,
        ):
            ht = cpool.tile([P, KC1, T], F16, tag="ht")
            # fp16 ping-pong expert accumulators (DVE in-place add on one
            # buffer is illegal -> read acc[e%2], write acc[(e+1)%2])
            acc_a = cpool.tile([P, NSUB * D], F16, tag="acc_a")
            acc_b = cpool.tile([P, NSUB * D], F16, tag="acc_b")
            accs = [acc_a, acc_b]
            b1_sb = cpool.tile([P, E * MC1], F32, tag="b1")
            b2_sb = cpool.tile([P, D], F32, tag="b2")

            # expert 0 weights at slab granularity (fast head: first matmul
            # only needs slab k=0 + ht chunk 0), interleaved with ht chunks
            w1_t = [None] * E
            w2_t = [None] * E
            w1_t[0] = w1_pool.tile([P, KC1, H], F16, tag="w1", name="w1_0")
            for k in range(KC1):
                nc.sync.dma_start(ht[:, k:k + 1, :], ht3[:, k:k + 1, :])
                nc.sync.dma_start(w1_t[0][:, k:k + 1, :],
                                  w13[:, k:k + 1, :])
            nc.sync.dma_start(b1_sb[:], b1t[:])
            w2_t[0] = w2_pool.tile([P, MC1, D], F16, tag="w2", name="w2_0")
            for q in range(4):
                nc.sync.dma_start(
                    w2_t[0][:, 4 * q:4 * q + 4, :],
                    w23[:, 4 * q:4 * q + 4, :],
                )
            nc.sync.dma_start(b2_sb[:], b2s[:])

            def prefetch_w(e, w2_engine=None):
                w1_t[e] = w1_pool.tile([P, KC1, H], F16, tag="w1", name=f"w1_{e}")
                nc.sync.dma_start(w1_t[e][:],
                                  w13[:, e * KC1:(e + 1) * KC1, :])
                w2_t[e] = w2_pool.tile([P, MC1, D], F16, tag="w2", name=f"w2_{e}")
                (w2_engine or nc.scalar).dma_start(
                    w2_t[e][:], w23[:, e * MC1:(e + 1) * MC1, :])

            # expert 1 prefetch rides the SAME (sync) ring, queued behind all
            # of expert 0's weights: single-ring FIFO is exactly the right
            # priority order, and an idle scalar ring can't head-of-line
            # block expert 0's gelu ACTs behind a stalled DMA issue.
            prefetch_w(1, w2_engine=nc.sync)

            for e in range(E):
                # --- layer 1: hidT chunk m = (W1 blk).T @ hT, +b1, gelu ---
                # Expert 0 runs k-outer over m-groups of 3 so the first
                # matmuls consume W1 slabs at their DMA arrival cadence
                # (m-outer would need all 8 slabs within the first chain).
                hid = hid_pool.tile([P, MC1, T], F16, tag="hid")

                def l1_act(m, ps):
                    nc.scalar.activation(
                        hid[:, m, :],
                        ps[:],
                        mybir.ActivationFunctionType.Gelu,
                        bias=b1_sb[:, e * MC1 + m:e * MC1 + m + 1],
                        scale=1.0,
                    )

                if e == 0:
                    for m0 in range(0, MC1, 3):
                        ms = list(range(m0, min(m0 + 3, MC1)))
                        pss_l1 = [
                            l1_psum.tile([P, T], F32, tag="l1",
                                         name=f"l1ps_{m0}_{i}")
                            for i in range(len(ms))]
                        for k in range(KC1):
                            for mi, m in enumerate(ms):
                                nc.tensor.matmul(
                                    pss_l1[mi][:],
                                    w1_t[e][:, k, m * P:(m + 1) * P],
                                    ht[:, k, :],
                                    start=(k == 0),
                                    stop=(k == KC1 - 1),
                                )
                        for mi, m in enumerate(ms):
                            l1_act(m, pss_l1[mi])
                else:
                    for m in range(MC1):
                        ps = l1_psum.tile([P, T], F32, tag="l1")
                        for k in range(KC1):
                            nc.tensor.matmul(
                                ps[:],
                                w1_t[e][:, k, m * P:(m + 1) * P],
                                ht[:, k, :],
                                start=(k == 0),
                                stop=(k == KC1 - 1),
                            )
                        l1_act(m, ps)

                if 1 <= e < E - 1:
                    prefetch_w(e + 1)

                # --- layer 2, accumulated into fe_acc over experts ---
                for s in range(NSUB):
                    ps_a = l2_psum.tile([P, 512], F32, tag="l2")
                    ps_b = l2_psum.tile([P, 512], F32, tag="l2")
                    pss = [ps_a, ps_b]
                    for m in range(MC1):
                        hs = hid[:, m, s * P:(s + 1) * P]
                        for d in range(ND):
                            mi = nc.tensor.matmul(
                                pss[d][:],
                                hs,
                                w2_t[e][:, m, d * 512:(d + 1) * 512],
                                start=(m == 0),
                                stop=(m == MC1 - 1),
                            )
                            if d > 0:
                                mi.ins.ldweights = False
                    if e == E - 1:
                        # last expert: single add (b2 was folded in at e=0),
                        # d-slices on separate engines, store immediately
                        osb = out_pool.tile([P, D], F16, tag="osb")
                        for d in range(ND):
                            lo = s * D + d * 512
                            nc.vector.tensor_add(
                                osb[:, d * 512:(d + 1) * 512], pss[d][:],
                                accs[e % 2][:, lo:lo + 512])
                        nc.sync.dma_start(out[s * P:(s + 1) * P, :], osb[:])
                    else:
                        for d in range(ND):
                            lo, hi = s * D + d * 512, s * D + (d + 1) * 512
                            if e == 0:
                                # fold the (gate-weighted) b2 in here so the
                                # final expert needs only one add
                                nc.vector.tensor_add(
                                    accs[1][:, lo:hi], pss[d][:],
                                    b2_sb[:, d * 512:(d + 1) * 512])
                            else:
                                nc.vector.tensor_add(
                                    accs[(e + 1) % 2][:, lo:hi], pss[d][:],
                                    accs[e % 2][:, lo:hi])

    nc.finalize()
    return nc


def _get_nc():
    if "nc" not in _CACHE:
        _CACHE["nc"] = build()
    return _CACHE["nc"]


def _run(inputs, trace=False):
    from concourse.bass_utils import run_bass_kernel_spmd

    h = np.asarray(inputs["h"], dtype=np.float32)
    gate_logits = np.asarray(inputs["gate_logits"], dtype=np.float64)
    W1 = np.asarray(inputs["W1"], dtype=np.float32)
    b1 = np.asarray(inputs["b1"], dtype=np.float32)
    W2 = np.asarray(inputs["W2"], dtype=np.float32)
    b2 = np.asarray(inputs["b2"], dtype=np.float32)

    # gate: softmax over E; fold into W2/b2
    z = np.exp(gate_logits - gate_logits.max())
    probs = (z / z.sum()).astype(np.float32)

    # w13[p, e*KC1+k, :] = W1[e, k*128+p, :]
    w13 = np.ascontiguousarray(
        W1.reshape(E * KC1, P, H).transpose(1, 0, 2).astype(np.float16)
    )
    w23 = np.ascontiguousarray(
        (W2 * probs[:, None, None]).reshape(E * MC1, P, D)
        .transpose(1, 0, 2).astype(np.float16)
    )
    # b1t[p, e, m] = b1[e, m*128+p]
    b1t = np.ascontiguousarray(
        b1.reshape(E, MC1, P).transpose(2, 0, 1).reshape(P, E * MC1)
        .astype(np.float32)
    )
    b2sum = (probs[:, None] * b2).sum(axis=0).astype(np.float32)
    b2s = np.ascontiguousarray(np.broadcast_to(b2sum, (P, D)))

    in_maps = []
    for r in range(NCORES):
        hr = h[r * T:(r + 1) * T, :]                      # [T, IN]
        ht3 = np.ascontiguousarray(
            hr.T.reshape(KC1, P, T).transpose(1, 0, 2).astype(np.float16)
        )
        in_maps.append(
            {"ht3": ht3, "w13": w13, "b1t": b1t, "w23": w23, "b2s": b2s}
        )

    nc = _get_nc()
    res = run_bass_kernel_spmd(nc, in_maps, list(range(NCORES)), trace=trace)

    final = np.concatenate(
        [np.asarray(res.results[r]["out"], dtype=np.float32)
         for r in range(NCORES)], axis=0
    )
    return final, res


def kernel(**inputs):
    final, _ = _run(inputs, trace=False)
    return final
